# revision 23
# baseline (speedup 1.0000x reference)
"""GAT layer kernel for Trainium2, 8 NeuronCores (SPMD) — v3.

Strategy (edge-sharded by dst, node table replicated, core-local node
numbering):
  Host: each core m renumbers nodes local = (v - m*6272) mod 50176, so
  core m's own 98 dst-slots (64 nodes each) sit at local ids [0, 6272).
  All per-core DRAM addresses are then core-independent and the single
  SPMD program works unmodified on every core; only the inputs (rolled
  hT/embT, per-core edge indices) differ.

  Phase A (each core, replicated over its local numbering):
    z = h @ W_lin.T plus per-node score scalars a_src/a_dst; writes
      table_lo/table_hi rows: [z(128) | 1.0 | a_src | pad] bf16, 512B
      adst_d: compact [50176, 1] bf16 of per-node a_dst
    lo rows (local id < 32768) are written first so phase-B lo gathers
    can start while hi rows are still being produced.
  Phase B per 64-node dst-slot (C ~17 chunks of 128 edges):
    - dma_gather table rows by edge src (int16 local idx, lo/hi halves)
    - ad_row [1,64] <- adst_d[k*64] (one 128B descriptor), replicated
      across partitions by a K=1 PE matmul -> adT [128,64]
    - per-edge scores on [P,C,64]: w = exp(lrelu(asrc_bc + adT_bc)),
      mask-select S = (dl_bc == iota64) * w   (3 DVE + 2 ACT passes)
    - per chunk ONE matmul ps[64,129] += S_c^T @ [z | 1] accumulating
      numerator and denominator together.
    - out tile = ps[:,0:128] * reciprocal(ps[:,128])
Host gathers per-core output slots back into the full [N, 128] array.
"""

import os
import sys
import types
import numpy as np

N = 50000
E = 1600000
D = 128
P = 128                              # partitions / edge-chunk size
T64 = 64                             # dst-tile width (nodes per slot)
TSLOTS = 98                          # 64-slots per core
CHUNK_N = TSLOTS * T64               # 6272 nodes per core
NNODES_PAD = 8 * CHUNK_N             # 50176 local id space
SPLIT = 32768                        # int16 gather index limit
NEG_SLOPE = 0.01
PAD_DSTLOC = 200.0                   # sentinel: never matches iota 0..63
MAX_GIDX = 512                       # dma_gather idxs/call
NQUEUES = 4

RLEN = 256                           # row elems (bf16) -> 512B rows
ONE_OFF = 128                        # constant 1.0 column
ASRC_OFF = 129                       # a_src column
ROW_W = 130                          # row elems written

LAST_EXEC_NS = None

# ---------------------------------------------------------------- toolchain fixes


def _apply_tilefix():
    import concourse.tile as tile_mod
    from concourse._compat import not_none as nn
    from concourse.vector_clock import ScopedClock

    def _patched_drain_and_barrier(self, tick_clock, wait_clock):
        nc = self.nc
        probe = nc.sync.nop()
        wait_clock.add_sem_waits(
            probe.ins, ScopedClock({None: tick_clock.global_clock}))
        si = probe.ins.sync_info
        waits = list(si.on_wait) if si is not None and si.on_wait else []
        nn(nc.cur_bb).bb.instructions.remove(probe.ins)
        by_name = {h.name: h for h in self.sems.allocated().values()}
        for w in waits:
            h = by_name[w.ant_name]
            assert w.wait_mode == "sem-ge-imm", w.wait_mode
            nc.sync.wait_ge(h, w.wait_value)
        nc.sync.drain()
        nc.all_engine_barrier()
        assert self.sems is not None
        popped = nc._tile_sem_poison_stack.pop()
        assert popped is self._sem_poison
        nc.clear_and_free_semaphores(list(self.sems.allocated().values()))
        nc.all_engine_barrier()

    tile_mod.TileContext._drain_and_barrier = _patched_drain_and_barrier


def _legalize_waits(nc):
    """This container's walrus caps sync waits at 1 per instruction; hoist
    extras onto standalone EventSemaphore (wait) instructions."""
    import concourse.mybir as mybir
    MAXW = 1
    for f in nc.m.functions:
        for bb in f.blocks:
            insts = bb.instructions
            new_list = []
            changed = False
            for ins in list(insts):
                si = ins.sync_info
                waits = list(si.on_wait) if (si is not None and si.on_wait) else []
                if len(waits) > MAXW:
                    changed = True
                    extra, keep = waits[:-MAXW], waits[-MAXW:]
                    for j in range(0, len(extra), MAXW):
                        chunk = extra[j:j + MAXW]
                        ev = mybir.InstEventSemaphore(
                            name=f"{ins.name}-waitfix{j}", ins=[], outs=[])
                        ev.engine = ins.engine
                        ev.sync_info = mybir.SyncInfo(on_wait=chunk, on_update=[])
                        new_list.append(ev)
                    si.on_wait = keep
                new_list.append(ins)
            if changed:
                bb.instructions = new_list


def _apply_profhook():
    try:
        import antenv.axon_hooks  # noqa: F401
        return
    except ImportError:
        pass
    try:
        from trn_agent_boot.trn_boot import _ntff_profile_via_ctypes
        hook = _ntff_profile_via_ctypes('/opt/axon/libaxon_pjrt.so')
    except Exception:
        hook = None
    mod = types.ModuleType('antenv.axon_hooks')
    mod._hook = hook
    mod.get_axon_ntff_profile_hook = lambda: mod._hook
    mod.set_axon_ntff_profile_hook = lambda h: setattr(mod, '_hook', h)
    sys.modules['antenv.axon_hooks'] = mod


# ---------------------------------------------------------------- host prep


def _ceil_div(a, b):
    return -(-a // b)


def _wrap_idx(arr):
    """[n] int16 -> [128, n/16] wrapped-in-16-partitions, replicated x8."""
    a = np.asarray(arr, dtype=np.int16)
    assert a.size % 16 == 0
    w = a.reshape(-1, 16).T.copy()            # [16, n/16]
    return np.tile(w, (8, 1))                 # [128, n/16]


def _prepare(src, dst):
    """Sort edges by dst; build per-core, per-slot chunk schedules and index
    arrays (in core-local node numbering). Returns (schedule, per_core)."""
    src = np.asarray(src).astype(np.int64)
    dst = np.asarray(dst).astype(np.int64)
    order = np.argsort(dst, kind="stable")
    s_s = src[order]
    d_s = dst[order]

    tile_start = np.searchsorted(d_s, np.arange(0, NNODES_PAD + T64, T64))
    n_lo = np.zeros((8, TSLOTS), np.int64)
    n_hi = np.zeros((8, TSLOTS), np.int64)
    edges_lo = {}
    edges_hi = {}
    for m in range(8):
        for k in range(TSLOTS):
            t = m * TSLOTS + k
            e0, e1 = tile_start[t], tile_start[t + 1]
            ls = (s_s[e0:e1] - m * CHUNK_N) % NNODES_PAD   # local src ids
            dl = d_s[e0:e1] - t * T64
            lo = ls < SPLIT
            sl, dll = ls[lo], dl[lo]
            o = np.argsort(sl, kind="stable")
            edges_lo[(m, k)] = (sl[o], dll[o])
            sh, dlh = ls[~lo] - SPLIT, dl[~lo]
            o = np.argsort(sh, kind="stable")
            edges_hi[(m, k)] = (sh[o], dlh[o])
            n_lo[m, k] = int(lo.sum())
            n_hi[m, k] = int((~lo).sum())

    C_lo = [max(1, _ceil_div(int(n_lo[:, k].max()), P)) for k in range(TSLOTS)]
    C_hi = [max(1, _ceil_div(int(n_hi[:, k].max()), P)) for k in range(TSLOTS)]

    import ml_dtypes
    bf = ml_dtypes.bfloat16
    per_core = []
    for m in range(8):
        idx_cols = []      # int16 wrapped cols, concatenated along axis 1
        dstloc_cols = []   # [P, C] bf16 per slot
        for k in range(TSLOTS):
            parts = []
            for (edges, C) in ((edges_lo[(m, k)], C_lo[k]),
                               (edges_hi[(m, k)], C_hi[k])):
                ss, dl = edges
                n = C * P
                idx = np.zeros(n, np.int64)
                dlc = np.full(n, PAD_DSTLOC, np.float32)
                idx[: ss.size] = ss
                dlc[: dl.size] = dl.astype(np.float32)
                parts.append((idx, dlc, C))
            slot_dl = []
            for idx, dlc, C in parts:
                j = 0
                while j < idx.size:
                    n_sub = min(MAX_GIDX, idx.size - j)
                    idx_cols.append(_wrap_idx(idx[j:j + n_sub]))
                    j += n_sub
                slot_dl.append(dlc.reshape(C, P).T)     # [P, C]
            dstloc_cols.append(np.concatenate(slot_dl, axis=1))
        idx_all = np.concatenate(idx_cols, axis=1).astype(np.int16)  # [P, sum]
        dstloc_all = np.concatenate(dstloc_cols, axis=1).astype(bf)
        per_core.append({"idx": idx_all, "dstloc": dstloc_all})

    schedule = (tuple(C_lo), tuple(C_hi))
    return schedule, per_core


# ---------------------------------------------------------------- device program

_BUILD_CACHE = {}


def _build(schedule, idx_width):
    import concourse.bass as bass  # noqa: F401
    import concourse.mybir as mybir
    import concourse.tile as tile
    from concourse import bacc, library_config

    C_lo, C_hi = schedule
    C_tot = [a + b for a, b in zip(C_lo, C_hi)]

    nc = bacc.Bacc("TRN2", dynamic_dma_scratch_size=131072,
                   num_swdge_queues=NQUEUES)
    f32 = mybir.dt.float32
    i16 = mybir.dt.int16
    bf16 = mybir.dt.bfloat16

    NT128 = NNODES_PAD // P              # 392 tiles of 128 for phase A
    LO_T128 = SPLIT // P                 # 256: tiles below SPLIT

    hT = nc.dram_tensor("hT", [P, NNODES_PAD], bf16, kind="ExternalInput")
    embT = nc.dram_tensor("embT", [P, NNODES_PAD], bf16, kind="ExternalInput")
    W_linT = nc.dram_tensor("W_linT", [P, P], bf16, kind="ExternalInput")
    W_lin = nc.dram_tensor("W_lin", [P, P], bf16, kind="ExternalInput")
    wfc = nc.dram_tensor("wfc", [P, 2], bf16, kind="ExternalInput")
    wemb = nc.dram_tensor("wemb", [P, 2], bf16, kind="ExternalInput")
    iota_in = nc.dram_tensor("iota_in", [P, P], bf16, kind="ExternalInput")
    ident_in = nc.dram_tensor("ident_in", [P, P], bf16, kind="ExternalInput")
    idx_in = nc.dram_tensor("idx_in", [P, idx_width], i16, kind="ExternalInput")
    dstloc_in = nc.dram_tensor("dstloc_in", [P, sum(C_tot)], bf16,
                               kind="ExternalInput")

    table_lo = nc.dram_tensor("table_lo", [SPLIT, RLEN], bf16)
    table_hi = nc.dram_tensor("table_hi", [NNODES_PAD - SPLIT, RLEN], bf16)
    adst_d = nc.dram_tensor("adst_d", [NNODES_PAD // P, P], bf16)
    out_d = nc.dram_tensor("out", [TSLOTS * T64, P], f32, kind="ExternalOutput")

    with tile.TileContext(nc) as tc:
        with tc.tile_pool(name="const", bufs=1) as cpool:
            nc.gpsimd.load_library(library_config.mlp)
            iota_t = cpool.tile([P, P], bf16)
            ident_t = cpool.tile([P, P], bf16)
            rh1 = cpool.tile([P, 130], bf16)       # [W_linT | u1 | u2]
            wemb_t = cpool.tile([P, 2], bf16)
            wl_t = cpool.tile([P, P], bf16)
            wfc_t = cpool.tile([P, 2], bf16)
            ones_row = cpool.tile([1, P], bf16)
            adst_sb = cpool.tile([P, 512], bf16)   # a_dst column per 128-tile
            nc.sync.dma_start(out=iota_t[:], in_=iota_in[:])
            nc.sync.dma_start(out=ident_t[:], in_=ident_in[:])
            nc.sync.dma_start(out=rh1[:, 0:P], in_=W_linT[:])
            nc.sync.dma_start(out=wemb_t[:], in_=wemb[:])
            nc.sync.dma_start(out=wl_t[:], in_=W_lin[:])
            nc.sync.dma_start(out=wfc_t[:], in_=wfc[:])
            nc.vector.memset(ones_row[:], 1.0)
            nc.vector.memset(adst_sb[:], 0.0)

            # ---- u1/u2 = W_lin.T @ w_fc halves -> rh1[:, 128:130]
            with tc.tile_pool(name="upsum", bufs=1, space="PSUM") as upp:
                ups = upp.tile([P, 2], f32, space="PSUM")
                nc.tensor.matmul(ups[:], lhsT=wl_t[:], rhs=wfc_t[:],
                                 start=True, stop=True)
                nc.vector.tensor_copy(out=rh1[:, P:P + 2], in_=ups[:])

            # ---- Phase A + B share one pool scope so the scheduler can
            # overlap them; explicit fences order gathers after table writes.
            from concourse.tile import add_dep_helper
            lo_stores = []
            hi_stores = []
            ad_stores = []
            LCH = 8
            SCH = 4
            with tc.tile_pool(name="pa", bufs=2) as pa, \
                 tc.tile_pool(name="pas", bufs=3) as pas, \
                 tc.tile_pool(name="pap", bufs=2, space="PSUM") as pap, \
                 tc.tile_pool(name="pb", bufs=3) as pb, \
                 tc.tile_pool(name="pbs", bufs=3) as pbs, \
                 tc.tile_pool(name="pbw", bufs=2) as pbw, \
                 tc.tile_pool(name="pbp", bufs=2, space="PSUM") as pbp, \
                 tc.tile_pool(name="pbp2", bufs=2, space="PSUM") as pbp2:
                for t0 in range(0, NT128, LCH):
                    nt = min(LCH, NT128 - t0)
                    hch = pa.tile([P, LCH * P], bf16, tag="hch")
                    ech = pa.tile([P, LCH * P], bf16, tag="ech")
                    nc.sync.dma_start(
                        out=hch[:, 0:nt * P], in_=hT[:, t0 * P:(t0 + nt) * P])
                    nc.sync.dma_start(
                        out=ech[:, 0:nt * P], in_=embT[:, t0 * P:(t0 + nt) * P])
                    for s0 in range(0, nt, SCH):
                        ns = min(SCH, nt - s0)
                        row4 = pas.tile([P, SCH, ROW_W], bf16, tag="row4")
                        for i in range(ns):
                            t = t0 + s0 + i
                            lh = hch[:, (s0 + i) * P:(s0 + i + 1) * P]
                            le = ech[:, (s0 + i) * P:(s0 + i + 1) * P]
                            ps_a = pap.tile([P, 130], f32, space="PSUM",
                                            tag="ps_a")
                            nc.tensor.matmul(ps_a[:], lhsT=lh, rhs=rh1[:],
                                             start=True, stop=False)
                            nc.tensor.matmul(ps_a[:, P:P + 2], lhsT=le,
                                             rhs=wemb_t[:],
                                             start=False, stop=True)
                            rw = row4[:, i, :]
                            nc.scalar.copy(out=rw[:, 0:P], in_=ps_a[:, 0:P])
                            nc.vector.memset(rw[:, ONE_OFF:ONE_OFF + 1], 1.0)
                            nc.vector.tensor_copy(
                                out=rw[:, ASRC_OFF:ASRC_OFF + 1],
                                in_=ps_a[:, P:P + 1])
                            nc.vector.tensor_copy(
                                out=adst_sb[:, t:t + 1],
                                in_=ps_a[:, P + 1:P + 2])
                        tt = t0 + s0
                        if tt + ns <= LO_T128:
                            tdst = table_lo[tt * P:(tt + ns) * P, 0:ROW_W]
                        else:
                            tdst = table_hi[tt * P - SPLIT:(tt + ns) * P - SPLIT,
                                            0:ROW_W]
                        st = nc.sync.dma_start(
                            out=tdst.rearrange("(s p) e -> p s e", s=ns),
                            in_=row4[:, 0:ns, :])
                        (lo_stores if tt + ns <= LO_T128
                         else hi_stores).append(st.ins)
                # transpose a_dst columns so each partition holds one 128-tile
                # (256B contiguous) and store node-major to adst_d.
                for j in range(4):
                    nrows = min(P, NT128 - j * P)
                    if nrows <= 0:
                        break
                    ps_t = pap.tile([P, P], f32, space="PSUM", tag="ps_t")
                    nc.tensor.matmul(ps_t[:],
                                     lhsT=adst_sb[:, j * P:(j + 1) * P],
                                     rhs=ident_t[:], start=True, stop=True)
                    adT_j = pas.tile([P, P], bf16, tag="adT_j")
                    nc.scalar.copy(out=adT_j[0:nrows, :], in_=ps_t[0:nrows, :])
                    st = nc.sync.dma_start(
                        out=adst_d[j * P:j * P + nrows, :],
                        in_=adT_j[0:nrows, :])
                    ad_stores.append(st.ins)

                # fences: gathers wait for the matching table half; ad_row
                # loads wait for adst_d.
                fence_lo = nc.sync.nop()
                fence_hi = nc.sync.nop()
                fence_ad = nc.sync.nop()
                for s in lo_stores:
                    add_dep_helper(fence_lo.ins, s, reason="table_lo fence")
                for s in hi_stores:
                    add_dep_helper(fence_hi.ins, s, reason="table_hi fence")
                for s in ad_stores:
                    add_dep_helper(fence_ad.ins, s, reason="adst_d fence")

                # ---- Phase B: per 64-node dst-slot
                idx_off = 0
                dl_off = 0
                gq = 0
                for k in range(TSLOTS):
                    C = C_tot[k]
                    gbuf = pb.tile([P, C, RLEN], bf16, tag="gbuf")
                    wk = C * P // 16
                    it = pbs.tile([P, wk], i16, tag="idx")
                    nc.sync.dma_start(
                        out=it[:], in_=idx_in[:, idx_off:idx_off + wk])
                    idx_off += wk
                    iw = 0
                    for (Ch, base) in ((C_lo[k], 0), (C_hi[k], SPLIT)):
                        ntot = Ch * P
                        cpos = 0 if base == 0 else C_lo[k]
                        j = 0
                        while j < ntot:
                            n_sub = min(MAX_GIDX, ntot - j)
                            w16 = n_sub // 16
                            c0 = cpos + j // P
                            nsc = n_sub // P
                            src_ap = table_lo[:] if base == 0 else table_hi[:]
                            g = nc.gpsimd.dma_gather(
                                out_ap=gbuf[:, c0:c0 + nsc, :], in_ap=src_ap,
                                idxs_ap=it[:, iw:iw + w16], num_idxs=n_sub,
                                num_idxs_reg=n_sub, elem_size=RLEN,
                                queue_num=gq % NQUEUES)
                            fence = fence_lo if base == 0 else fence_hi
                            add_dep_helper(getattr(g, "ins", g), fence.ins,
                                           reason="gather after table fence")
                            gq += 1
                            iw += w16
                            j += n_sub

                    # a_dst row for this slot: one 128B descriptor, then
                    # replicate across partitions with a K=1 matmul.
                    ad_row = pbs.tile([1, T64], bf16, tag="ad_row")
                    adl = nc.sync.dma_start(
                        out=ad_row[:],
                        in_=adst_d[k // 2:k // 2 + 1,
                                   T64 * (k % 2):T64 * (k % 2) + T64])
                    add_dep_helper(getattr(adl, "ins", adl), fence_ad.ins,
                                   reason="ad_row after adst_d fence")
                    ps_ar = pbp2.tile([P, T64], f32, space="PSUM", tag="ps_ar")
                    nc.tensor.matmul(ps_ar[:], lhsT=ones_row[:], rhs=ad_row[:],
                                     start=True, stop=True)
                    adT = pbs.tile([P, T64], bf16, tag="adT")
                    nc.scalar.copy(out=adT[:], in_=ps_ar[:])

                    # dst_local columns for this slot
                    dl_t = pbs.tile([P, C], bf16, tag="dl_t")
                    nc.sync.dma_start(
                        out=dl_t[:], in_=dstloc_in[:, dl_off:dl_off + C])
                    dl_off += C

                    # scores w[p,c,j] = exp(lrelu(a_src[p,c] + adT[p,j]))
                    w_t = pbw.tile([P, C, T64], bf16, tag="w_t")
                    asrc_bc = (gbuf[:, :, ASRC_OFF:ASRC_OFF + 1]
                               .broadcast_to([P, C, T64]))
                    adt_bc = adT[:].unsqueeze(1).broadcast_to([P, C, T64])
                    nc.vector.tensor_tensor(
                        out=w_t[:], in0=asrc_bc, in1=adt_bc,
                        op=mybir.AluOpType.add)
                    wflat = w_t[:].rearrange("p c j -> p (c j)")
                    nc.scalar.activation(
                        out=wflat, in_=wflat,
                        func=mybir.ActivationFunctionType.Prelu,
                        bias=0.0, scale=1.0, alpha=NEG_SLOPE)
                    nc.scalar.activation(
                        out=wflat, in_=wflat,
                        func=mybir.ActivationFunctionType.Exp)

                    # mask + select: S = (dl == iota64) * w
                    s_t = pbw.tile([P, C, T64], bf16, tag="s_t")
                    dl_bc = dl_t[:].unsqueeze(2).broadcast_to([P, C, T64])
                    iota_bc = (iota_t[:, 0:T64].unsqueeze(1)
                               .broadcast_to([P, C, T64]))
                    nc.vector.tensor_tensor(
                        out=s_t[:], in0=dl_bc, in1=iota_bc,
                        op=mybir.AluOpType.is_equal)
                    nc.vector.tensor_tensor(
                        out=s_t[:], in0=s_t[:], in1=w_t[:],
                        op=mybir.AluOpType.mult)

                    # routing matmuls: ps[:, 0:129] += S_c^T @ [z | 1]
                    ps_nd = pbp.tile([T64, P + 1], f32, space="PSUM",
                                     tag="ps_nd")
                    for c in range(C):
                        nc.tensor.matmul(ps_nd[:], lhsT=s_t[:, c, :],
                                         rhs=gbuf[:, c, 0:P + 1],
                                         start=(c == 0), stop=(c == C - 1))

                    den_s = pbs.tile([T64, 1], f32, tag="den_s")
                    nc.vector.tensor_scalar(
                        out=den_s[:], in0=ps_nd[:, P:P + 1], scalar1=1e-30,
                        scalar2=None, op0=mybir.AluOpType.add)
                    den_r = pbs.tile([T64, 1], f32, tag="den_r")
                    nc.vector.reciprocal(out=den_r[:], in_=den_s[:])
                    o_t = pbs.tile([T64, P], f32, tag="o_t")
                    nc.vector.tensor_scalar(
                        out=o_t[:], in0=ps_nd[:, 0:P], scalar1=den_r[:, 0:1],
                        scalar2=None, op0=mybir.AluOpType.mult)
                    nc.sync.dma_start(
                        out=out_d[k * T64:(k + 1) * T64, :], in_=o_t[:])

    nc.compile()
    _legalize_waits(nc)
    return nc


# ---------------------------------------------------------------- entry point


def kernel(h, embedding, W_lin, w_fc, w_emb, src, dst):
    global LAST_EXEC_NS
    _apply_tilefix()
    _apply_profhook()
    from concourse import bass_utils

    h = np.asarray(h, dtype=np.float32)
    embedding = np.asarray(embedding, dtype=np.float32)
    W_lin = np.asarray(W_lin, dtype=np.float32)
    w_fc = np.asarray(w_fc, dtype=np.float32).reshape(-1)
    w_emb = np.asarray(w_emb, dtype=np.float32).reshape(-1)

    schedule, per_core = _prepare(src, dst)
    idx_width = per_core[0]["idx"].shape[1]
    key = (schedule, idx_width)
    if key not in _BUILD_CACHE:
        _BUILD_CACHE[key] = _build(schedule, idx_width)
    nc = _BUILD_CACHE[key]

    import ml_dtypes
    bf = ml_dtypes.bfloat16
    hT_full = np.zeros((P, NNODES_PAD), bf)
    hT_full[:, :N] = h.T.astype(bf)
    embT_full = np.zeros((P, NNODES_PAD), bf)
    embT_full[:, :N] = embedding.T.astype(bf)
    iota_np = np.tile(np.arange(P)[None, :], (P, 1)).astype(bf)
    ident_np = np.eye(P).astype(bf)
    wfc_np = np.stack([w_fc[:D], w_fc[D:]], axis=1).astype(bf)
    wemb_np = np.stack([w_emb[:D], w_emb[D:]], axis=1).astype(bf)

    in_maps = []
    for m in range(8):
        in_maps.append({
            "hT": np.roll(hT_full, -m * CHUNK_N, axis=1),
            "embT": np.roll(embT_full, -m * CHUNK_N, axis=1),
            "W_lin": W_lin.astype(bf), "W_linT": W_lin.T.copy().astype(bf),
            "wfc": wfc_np, "wemb": wemb_np,
            "iota_in": iota_np, "ident_in": ident_np,
            "idx_in": per_core[m]["idx"],
            "dstloc_in": per_core[m]["dstloc"],
        })

    trace = os.environ.get("GAT_TRACE", "0") == "1"
    res = bass_utils.run_bass_kernel_spmd(
        nc, in_maps, core_ids=list(range(8)), trace=trace)
    LAST_EXEC_NS = res.exec_time_ns

    out = np.zeros((NNODES_PAD, P), np.float32)
    for m in range(8):
        out[m * CHUNK_N:(m + 1) * CHUNK_N] = res.results[m]["out"]
    return out[:N]
